# revision 13
# baseline (speedup 1.0000x reference)
"""Masked dot-product attention (B=8, Q=K=2048, D=512) on 8 trn2 NeuronCores.

Strategy: valid-length skipping + cross-batch load balancing.

The reference replaces masked scores with 0 before the softmax, so a masked
key k >= L gets weight exp(0)=1 and still contributes v_k to the numerator
and 1 to the denominator. Hence for any per-(batch,q-chunk) "slot" that only
processes keys [0, 128*cap):
    numerator += sum_{k >= 128*cap} v_k      (host-known constant vector)
    Z         += 2048 - 128*cap              (compile-time constant)
The vector part is folded into the masked rows of V (whose zeroed K^T columns
give them weight exactly 1), so the device needs no extra correction matmuls.

Sharding: batches are snake-assigned to two classes of 4 cores each; within a
class, each core takes a 512-query slice of each of its 4 batches. All cores
run the SAME program (SPMD) built from a shared "profile" of per-group k-tile
caps (elementwise max over the two classes); only the host-packed input data
differs per core. With seed-0 lens the per-core work is 152 k-tile-chunks vs
256 for the batch-per-core baseline.

Per core, groups are processed smallest-cap first (fast pipeline start):
  phase 1 (per group, per k-tile): S^T[128k, 512q] = K^T_tile^T @ Q^T via 4
    contraction chunks into one PSUM bank; ScalarE exp -> X bf16.
  phase 2 (per 128-q slot): O[128q,512] accumulates X^T-tile @ V-tile over the
    cap tiles; Z via a ones matmul sharing the stationary operand; VectorE
    adds the masked-count constant, reciprocal, scale, fp32 out.
"""

import sys

if "/opt/trn_rl_repo" not in sys.path:
    sys.path.insert(0, "/opt/trn_rl_repo")

import numpy as np
import ml_dtypes

BF16 = ml_dtypes.bfloat16

B, SEQ, D = 8, 2048, 512
P = 128
ND = D // P            # 4 contraction chunks of the d dimension
NCLS = 2               # core classes
CPC = B // NCLS        # batches per class (4)
QSL = SEQ // CPC       # query slice per core per batch (512)
NSL = QSL // P         # 128-q slots per group (4)
SCALE = 1.0 / float(np.sqrt(D))
ZN = 8                 # ones width for the Z matmul

_CACHE = {}


def _plan(valid_lens):
    """Snake-assign batches to NCLS classes; shared profile caps."""
    lens = [int(v) for v in valid_lens]
    # ceil(L/128); for L an exact tile multiple short of SEQ, add one tile so
    # the slot always has at least one masked row to carry the tail-sum fold.
    tiles = [max(1, -(-(l + (1 if (l % P == 0 and l < SEQ) else 0)) // P))
             for l in lens]
    order = sorted(range(B), key=lambda b: -tiles[b])
    classes = [[] for _ in range(NCLS)]
    for i, b in enumerate(order):
        # snake: A,B,B,A,A,B,B,A ...
        k = i % (2 * NCLS)
        cls = k if k < NCLS else 2 * NCLS - 1 - k
        classes[cls].append(b)  # stays sorted desc by tiles
    # caps[i] = max over classes of i-th largest tile count
    caps = [max(tiles[cls[i]] for cls in classes) for i in range(CPC)]
    # Group order: the two smallest first (cheap DMA fills the pipe while the
    # PE spins up), then the largest (long compute hides the remaining input
    # stream), then the rest descending.
    asc = sorted(range(CPC), key=lambda i: caps[i])
    gorder = asc[:2] + asc[:1:-1]
    caps = [caps[i] for i in gorder]
    classes = [[cls[i] for i in gorder] for cls in classes]
    return classes, caps, tiles


def _build(caps):
    import concourse.bacc as bacc
    import concourse.mybir as mybir
    from concourse.tile import TileContext

    nk = sum(caps)  # total k-tiles per core
    KC = nk * P     # total k columns

    nc = bacc.Bacc("TRN2")
    qt = nc.dram_tensor("qt", [P, ND, SEQ], mybir.dt.bfloat16, kind="ExternalInput")
    ktm = nc.dram_tensor("ktm", [P, ND, KC], mybir.dt.bfloat16, kind="ExternalInput")
    vm = nc.dram_tensor("vm", [P, nk, D], mybir.dt.bfloat16, kind="ExternalInput")
    out = nc.dram_tensor("out", [SEQ, D], mybir.dt.bfloat16, kind="ExternalOutput")

    FP32 = mybir.dt.float32
    BF = mybir.dt.bfloat16
    Exp = mybir.ActivationFunctionType.Exp

    with TileContext(nc) as tc:
        with tc.tile_pool(name="inp", bufs=1) as inp, \
             tc.tile_pool(name="xtp", bufs=2) as xtp, \
             tc.tile_pool(name="sp", bufs=2, space="PSUM") as sp, \
             tc.tile_pool(name="op", bufs=2, space="PSUM") as op, \
             tc.tile_pool(name="zp", bufs=2, space="PSUM") as zp, \
             tc.tile_pool(name="outp", bufs=4) as outp:

            ones = inp.tile([P, ZN], BF, name="ones")
            nc.vector.memset(ones, 1.0)

            # ---- input DMA (issue in group processing order) ----
            qts = inp.tile([P, ND, SEQ], BF, name="qts")
            kts = inp.tile([P, ND, KC], BF, name="kts")
            vts = inp.tile([P, nk, D], BF, name="vts")
            base = 0
            for g, cap in enumerate(caps):
                k0, k1 = base * P, (base + cap) * P
                q0 = g * QSL
                nc.sync.dma_start(kts[:, :, k0:k1], ktm[:, :, k0:k1])
                nc.sync.dma_start(qts[:, :, q0:q0 + QSL], qt[:, :, q0:q0 + QSL])
                nc.sync.dma_start(vts[:, base:base + cap, :],
                                  vm[:, base:base + cap, :])
                base += cap

            # ---- compute ----
            # Output staging: DMA-out instructions enqueue behind the input
            # stream on the descriptor queue, so they drain only after the
            # inputs — they never preempt a K/V transfer the PE is waiting
            # on. All 16 slots must therefore stay resident until then.
            osb = inp.tile([P, CPC * NSL, D], BF, name="osb")
            base = 0
            for g, cap in enumerate(caps):
                q0 = g * QSL
                x = xtp.tile([P, cap, QSL], BF, name=f"x{g}")
                for t in range(cap):
                    s_ps = sp.tile([P, QSL], FP32, name="s")
                    kcol = (base + t) * P
                    for d in range(ND):
                        nc.tensor.matmul(
                            s_ps,
                            lhsT=kts[:, d, kcol:kcol + P],
                            rhs=qts[:, d, q0:q0 + QSL],
                            start=(d == 0),
                            stop=(d == ND - 1),
                        )
                    nc.scalar.activation(x[:, t, :], s_ps, Exp, scale=SCALE)

                ct = float(SEQ - cap * P)
                for s in range(NSL):
                    qs = s * P
                    gs = g * NSL + s
                    opsum = op.tile([P, D], FP32, name="opsum")
                    zpsum = zp.tile([P, ZN], FP32, name="zpsum")
                    for t in range(cap):
                        w = x[:, t, qs:qs + P]
                        nc.tensor.matmul(
                            opsum, lhsT=w, rhs=vts[:, base + t, :],
                            start=(t == 0), stop=(t == cap - 1),
                        )
                        nc.tensor.matmul(
                            zpsum, lhsT=w, rhs=ones,
                            start=(t == 0), stop=(t == cap - 1),
                        )
                    zr = outp.tile([P, 1], FP32, name="zr")
                    if ct != 0.0:
                        zt = outp.tile([P, 1], FP32, name="zt")
                        nc.vector.tensor_scalar_add(zt, zpsum[:, 0:1], ct)
                        nc.vector.reciprocal(zr, zt)
                    else:
                        nc.vector.reciprocal(zr, zpsum[:, 0:1])
                    nc.vector.tensor_scalar_mul(osb[:, gs, :], opsum, zr)
                    nc.sync.dma_start(out[q0 + qs:q0 + qs + P, :], osb[:, gs, :])
                base += cap

    nc.compile()
    return nc


def _get_nc(caps):
    key = ("v2", tuple(caps))
    if key not in _CACHE:
        _CACHE[key] = _build(list(caps))
    return _CACHE[key]


def _prepare(queries, keys, values, valid_lens):
    """Per-core input packing. Returns (in_maps, plan)."""
    queries = np.asarray(queries, dtype=np.float32)
    keys = np.asarray(keys, dtype=np.float32)
    values = np.asarray(values, dtype=np.float32)
    lens = np.asarray(valid_lens).astype(np.int64)
    classes, caps, tiles = _plan(lens)
    nk = sum(caps)
    in_maps = []
    for cls in range(NCLS):
        batches = classes[cls]
        # shared per-class K^T / V blocks (same for the 4 cores of the class)
        ktm = np.zeros((D, nk * P), dtype=np.float32)
        vm = np.zeros((nk * P, D), dtype=np.float32)
        base = 0
        for g, cap in enumerate(caps):
            b = batches[g]
            L = int(lens[b])
            kc = cap * P
            cov = min(L, kc)
            ktm[:, base * P:base * P + cov] = keys[b, :cov, :].T
            vm[base * P:base * P + kc, :] = values[b, :kc, :]
            if kc > L:
                ts = values[b, kc:, :].sum(axis=0)
                vm[base * P + L:base * P + kc, :] += ts / (kc - L)
            base += cap
        # device layouts: ktm [128, ND, KC], vm [128, nk, D]
        ktm_d = np.ascontiguousarray(
            ktm.reshape(ND, P, nk * P).transpose(1, 0, 2)).astype(BF16)
        vm_d = np.ascontiguousarray(
            vm.reshape(nk, P, D).transpose(1, 0, 2)).astype(BF16)
        for r in range(CPC):
            qtm = np.empty((D, SEQ), dtype=np.float32)
            for g in range(CPC):
                b = batches[g]
                qtm[:, g * QSL:(g + 1) * QSL] = queries[b, r * QSL:(r + 1) * QSL, :].T
            qtm_d = np.ascontiguousarray(
                qtm.reshape(ND, P, SEQ).transpose(1, 0, 2)).astype(BF16)
            in_maps.append({"qt": qtm_d, "ktm": ktm_d, "vm": vm_d})
    return in_maps, (classes, caps)


def _unpack(outs, plan):
    classes, caps = plan
    full = np.empty((B, SEQ, D), dtype=np.float32)
    core = 0
    for cls in range(NCLS):
        batches = classes[cls]
        for r in range(CPC):
            o = outs[core]
            for g in range(CPC):
                b = batches[g]
                full[b, r * QSL:(r + 1) * QSL, :] = o[g * QSL:(g + 1) * QSL, :]
            core += 1
    return full


def kernel(queries, keys, values, valid_lens):
    from concourse import bass_utils

    in_maps, plan = _prepare(queries, keys, values, valid_lens)
    nc = _get_nc(plan[1])
    res = bass_utils.run_bass_kernel_spmd(
        nc, in_maps, core_ids=list(range(B)), trace=False
    )
    outs = [np.asarray(res.results[c]["out"]) for c in range(B)]
    return _unpack(outs, plan).astype(np.float32)
